# revision 28
# baseline (speedup 1.0000x reference)
"""Trainium2 Bass kernel for nn_LstmConv (GNN message passing + LSTMCell).

Sharding: dst nodes load-balanced across 8 cores (permuted into 49 tiles of
128 slots per core). Per core, edge-source rows are fetched from an HBM bf16
feat table with batched InstDMAGatherAnt calls (one lo/hi half-table pair per
tile group, spread over the 4 SWDGE queues so descriptor emission runs on all
8 Q7 cores). Segment-mean is a one-hot PE matmul per 128-edge chunk; the
LSTMCell runs as two PE matmuls + ACT/DVE epilogue per tile. Output is
written transposed and reassembled on the host.
"""

import sys, os

sys.path.insert(0, "/opt/trn_rl_repo")
sys.path.insert(0, os.path.dirname(os.path.abspath(__file__)))

import numpy as np
from ml_dtypes import bfloat16

N_NODES = 50000
N_EDGES = 800000
H = 128
MSG = 64
P = 128
NCORES = 8
TILES = 49
SLOTS = TILES * P          # 6272 per core
SPLIT = 32768              # int16 index range split for the gather table
NPAD = N_NODES + 4         # featT table rows (spare zero rows)
GBT = int(__import__("os").environ.get("KGBT", "2"))                    # tiles per gather group
NQ = int(__import__("os").environ.get("KNQ", "4"))                     # SWDGE queues

LAST_EXEC_NS = None
TRACE = False


def _wrap16(idx, width):
    """Pack linear index list into [128, width] (idx j at [j%16, j//16],
    replicated across the 8 gpsimd cores)."""
    out = np.zeros((P, width), np.uint16)
    n = len(idx)
    cols = (n + 15) // 16
    blk = np.zeros((16, width), np.uint16)
    flat = np.zeros(cols * 16, np.uint16)
    flat[:n] = np.asarray(idx).astype(np.int16).view(np.uint16)
    blk[:, :cols] = flat.reshape(cols, 16).T
    out[:] = np.tile(blk, (8, 1))
    return out


def _host_prep(feat, src0, dst0, src1, dst1, W_ih, W_hh, b_ih, b_hh):
    deg0 = np.bincount(dst0, minlength=N_NODES)
    deg1 = np.bincount(dst1, minlength=N_NODES)
    w = deg0 + deg1

    # snake-assign nodes (sorted by degree desc) into 392 tiles of <=128
    n_tiles_g = NCORES * TILES
    order = np.argsort(-w, kind="stable")
    tile_of_node = np.empty(N_NODES, np.int32)
    pos_in_tile = np.empty(N_NODES, np.int32)
    tcnt = np.zeros(n_tiles_g, np.int32)
    idx = 0
    fwd = True
    while idx < N_NODES:
        rng = range(n_tiles_g) if fwd else range(n_tiles_g - 1, -1, -1)
        for t in rng:
            if idx >= N_NODES:
                break
            if tcnt[t] < P:
                tile_of_node[order[idx]] = t
                pos_in_tile[order[idx]] = tcnt[t]
                tcnt[t] += 1
                idx += 1
        fwd = not fwd

    # balance tiles over cores by weight: snake over tiles sorted by weight
    tile_w = np.zeros(n_tiles_g, np.int64)
    np.add.at(tile_w, tile_of_node, w)
    torder = np.argsort(-tile_w, kind="stable")
    core_of_tile = np.empty(n_tiles_g, np.int32)
    tl_of_tile = np.empty(n_tiles_g, np.int32)
    k = 0
    fwd = True
    for rnd in range(TILES):
        cr = range(NCORES) if fwd else range(NCORES - 1, -1, -1)
        for c in cr:
            core_of_tile[torder[k]] = c
            tl_of_tile[torder[k]] = rnd
            k += 1
        fwd = not fwd

    core_of_node = core_of_tile[tile_of_node]
    slot_of_node = tl_of_tile[tile_of_node] * P + pos_in_tile  # slot within core

    # node_of_slot per core (-1 = ghost)
    node_of_slot = -np.ones((NCORES, SLOTS), np.int64)
    node_of_slot[core_of_node, slot_of_node] = np.arange(N_NODES)

    # per-node combined scales a_e = 1/max(cnt_e,1) * 1/max(has0+has1,1)
    has0 = (deg0 > 0).astype(np.float32)
    has1 = (deg1 > 0).astype(np.float32)
    invc = 1.0 / np.maximum(has0 + has1, 1.0)
    a0 = invc / np.maximum(deg0, 1.0)
    a1 = invc / np.maximum(deg1, 1.0)

    # per-core per-(tile, etype, half) edge groups (half: src<SPLIT / >=)
    groups = {}   # (core, tl, e, h) -> (srcs, slot_pos)
    for e, (src, dst) in enumerate(((src0, dst0), (src1, dst1))):
        c = core_of_node[dst]
        s = slot_of_node[dst]
        hl = (src >= SPLIT).astype(np.int64)
        key = (((c * TILES + s // P) * 2 + hl) * P + (s % P)).astype(np.int64)
        o = np.argsort(key, kind="stable")
        src_s, pp_s = src[o], (s % P)[o]
        gkey = ((c[o] * TILES + (s // P)[o]) * 2 + hl[o])
        bounds = np.searchsorted(gkey, np.arange(NCORES * TILES * 2 + 1))
        for g in range(NCORES * TILES * 2):
            lo, hi = bounds[g], bounds[g + 1]
            gg = g // 2
            groups[(gg // TILES, gg % TILES, e, g % 2)] = (src_s[lo:hi], pp_s[lo:hi])

    # common chunk counts per (tl, e, half): max over cores, in 128-chunks
    K = np.zeros((TILES, 2, 2), np.int32)
    for tl in range(TILES):
        for e in range(2):
            for hf in range(2):
                m = max(len(groups[(c, tl, e, hf)][0]) for c in range(NCORES))
                K[tl, e, hf] = max((m + 127) // 128, 1)

    # gather groups of GBT tiles; per group: lo call (all tl,e half=0) then hi.
    # Snake-pack tiles into groups by total chunk count so per-group chunk
    # sums are near-equal (minimizes the uniform-call-size padding).
    ngroups = (TILES + GBT - 1) // GBT
    tw = [(int(K[tl].sum()), tl) for tl in range(TILES)]
    tw.sort(reverse=True)
    gsum = [0] * ngroups
    gcnt = [0] * ngroups
    tg = [[] for _ in range(ngroups)]
    for wgt, tl in tw:
        best = min((g for g in range(ngroups) if gcnt[g] < GBT),
                   key=lambda g: gsum[g])
        tg[best].append(tl)
        gsum[best] += wgt
        gcnt[best] += 1
    tgroups = [sorted(g) for g in tg]

    # real-chunk order = for each tgroup: [lo chunks of (tl,e)...] +
    # [hi chunks of (tl,e)...]; doff col == global real-chunk idx.
    # All lo calls share num_idxs=NLO_COM*128 (one register), likewise hi;
    # the per-group shortfall is -1 idx padding, self-trimmed by the ucode.
    chunk_of = {}   # (tl, e, hf) -> (start_chunk, K)
    call_sizes = []  # per group: (n_real_lo_chunks, n_real_hi_chunks)
    cidx = 0
    for tls in tgroups:
        nlo = 0
        for tl in tls:
            for e in range(2):
                chunk_of[(tl, e, 0)] = (cidx, int(K[tl, e, 0]))
                cidx += K[tl, e, 0]
                nlo += int(K[tl, e, 0])
        nhi = 0
        for tl in tls:
            for e in range(2):
                chunk_of[(tl, e, 1)] = (cidx, int(K[tl, e, 1]))
                cidx += K[tl, e, 1]
                nhi += int(K[tl, e, 1])
        call_sizes.append((nlo, nhi))
    CTOT = cidx
    NLO_COM = max(n for n, _ in call_sizes)
    NHI_COM = max(n for _, n in call_sizes)

    # per-core data: idx16 (uniform-size call blocks), doff, scales, featloc
    per_core = []
    CALLW = (NLO_COM + NHI_COM) * 8   # wrapped idx cols per group
    WTOT = len(tgroups) * CALLW
    for c in range(NCORES):
        doff = np.full((P, CTOT), 255.0, np.float32)
        idxlin = np.zeros(CTOT * P, np.int64)
        for tl in range(TILES):
            for e in range(2):
                for hf in range(2):
                    srcs, pps = groups[(c, tl, e, hf)]
                    st, k = chunk_of[(tl, e, hf)]
                    n = len(srcs)
                    a = np.zeros(k * P, np.int64)
                    a[:n] = srcs - (SPLIT if hf else 0)
                    idxlin[st * P:(st + k) * P] = a
                    d = np.full(k * P, 255.0, np.float32)
                    d[:n] = pps
                    doff[:, st:st + k] = d.reshape(k, P).T
        idx16 = np.zeros((P, WTOT), np.int16)
        cbase = 0
        for gi, tls in enumerate(tgroups):
            nlo, nhi = call_sizes[gi]
            PADIDX = -1 if os.environ.get("KPAD", "neg") == "neg" else 0
            lo = np.full(NLO_COM * P, PADIDX, np.int64)
            lo[:nlo * P] = idxlin[cbase * P:(cbase + nlo) * P]
            hi = np.full(NHI_COM * P, PADIDX, np.int64)
            hi[:nhi * P] = idxlin[(cbase + nlo) * P:(cbase + nlo + nhi) * P]
            w0 = gi * CALLW
            idx16[:, w0:w0 + NLO_COM * 8] = _wrap16(
                lo.astype(np.int16), NLO_COM * 8)
            idx16[:, w0 + NLO_COM * 8:w0 + CALLW] = _wrap16(
                hi.astype(np.int16), NHI_COM * 8)
            cbase += nlo + nhi

        sc = np.zeros((P, TILES * 2), np.float32)
        av0 = np.where(node_of_slot[c] >= 0, a0[np.maximum(node_of_slot[c], 0)], 0.0)
        av1 = np.where(node_of_slot[c] >= 0, a1[np.maximum(node_of_slot[c], 0)], 0.0)
        for tl in range(TILES):
            sc[:, tl * 2] = av0[tl * P:(tl + 1) * P]
            sc[:, tl * 2 + 1] = av1[tl * P:(tl + 1) * P]
        sl = node_of_slot[c]
        floc = np.zeros((SLOTS, H), np.float32)
        floc[sl >= 0] = feat[sl[sl >= 0]]
        per_core.append(dict(idx16=idx16, doff=doff, scales=sc,
                             featloc=floc.T.astype(bfloat16).copy()))

    # feat table [NPAD, 128] bf16 (row 0 doubles as the pad row: its
    # contribution is killed by doff=255 -> hot col 0)
    featT = np.zeros((NPAD, H), np.float32)
    featT[:N_NODES] = feat
    featT = featT.astype(bfloat16)

    # gate order [i, f, g, o] (PyTorch native)
    wih = W_ih.T.astype(bfloat16).copy()              # [128, 256]
    whh = W_hh.T.astype(bfloat16).copy()              # [64, 256]
    bt = (b_ih + b_hh).astype(np.float32)
    biasT = np.stack([bt[:128], bt[128:]], axis=1).copy()  # [128, 2]
    iota = np.tile(np.arange(P, dtype=np.float32)[None, :], (P, 1))

    shared = dict(featT=featT, wih=wih, whh=whh, biasT=biasT, iota=iota)
    meta = (tgroups, chunk_of, call_sizes, CTOT, WTOT, NLO_COM, NHI_COM)
    return per_core, shared, node_of_slot, meta


_WS = [0]


def _split_multi_waits(nc, mybir, max_waits=1):
    """This container's walrus rejects >1 sync wait per instruction; split
    extra waits onto single-wait NoOps placed just before the instruction."""
    for fn in nc.m.functions:
        for bb in fn.blocks:
            new = []
            for ins in bb.instructions:
                si = ins.sync_info
                if si is not None and len(si.on_wait) > max_waits:
                    waits = list(si.on_wait)
                    for w in waits[:-max_waits]:
                        _WS[0] += 1
                        nop = mybir.InstNoOp(
                            name=f"I-waitsplit-{_WS[0]}", ins=[], outs=[]
                        )
                        nop.engine = ins.engine
                        nop.sync_info = mybir.SyncInfo(on_wait=[w], on_update=[])
                        new.append(nop)
                    si.on_wait = waits[-max_waits:]
                new.append(ins)
            bb.instructions[:] = new


def _build_nc(meta):
    from concourse import bass, mybir, tile
    from concourse.masks import make_identity
    from concourse.library_config import mlp
    from concourse.library_overlay import lower_extended_insts

    tgroups, chunk_of, call_sizes, CTOT, WTOT, NLO_COM, NHI_COM = meta
    CALLW = (NLO_COM + NHI_COM) * 8
    f32, bf16, i16 = mybir.dt.float32, mybir.dt.bfloat16, mybir.dt.int16
    nc = bass.Bass(num_swdge_queues=NQ)
    featT_d = nc.declare_dram_parameter("featT", [NPAD, H], bf16, isOutput=False)
    idx16_d = nc.declare_dram_parameter("idx16", [P, WTOT], i16, isOutput=False)
    doff_d = nc.declare_dram_parameter("doff", [P, CTOT], f32, isOutput=False)
    sc_d = nc.declare_dram_parameter("scales", [P, TILES * 2], f32, isOutput=False)
    wih_d = nc.declare_dram_parameter("wih", [P, 256], bf16, isOutput=False)
    whh_d = nc.declare_dram_parameter("whh", [64, 256], bf16, isOutput=False)
    bias_d = nc.declare_dram_parameter("biasT", [P, 2], f32, isOutput=False)
    iota_d = nc.declare_dram_parameter("iota", [P, P], f32, isOutput=False)
    floc_d = nc.declare_dram_parameter("featloc", [P, SLOTS], bf16, isOutput=False)
    outT_d = nc.declare_dram_parameter("outT", [P, SLOTS], f32, isOutput=True)

    GCMAX = NLO_COM + NHI_COM
    KMAX = max(chunk_of[(tl, e, 0)][1] + chunk_of[(tl, e, 1)][1]
               for tl in range(TILES) for e in range(2))

    with tile.TileContext(nc) as tc:
        with (
            tc.tile_pool(name="const", bufs=1) as cp,
            tc.tile_pool(name="gb", bufs=4) as gbp,
            tc.tile_pool(name="hot", bufs=3) as hp,
            tc.tile_pool(name="ep", bufs=2) as ep,
            tc.tile_pool(name="psm", bufs=2, space="PSUM") as psm,
            tc.tile_pool(name="pst", bufs=1, space="PSUM") as pst,
            tc.tile_pool(name="psg", bufs=1, space="PSUM") as psgp,
        ):
            idx16 = cp.tile([P, WTOT], i16)
            nc.sync.dma_start(out=idx16[:], in_=idx16_d[:])
            doff = cp.tile([P, CTOT], f32)
            nc.sync.dma_start(out=doff[:], in_=doff_d[:])
            sc = cp.tile([P, TILES * 2], f32)
            nc.sync.dma_start(out=sc[:], in_=sc_d[:])
            wih = cp.tile([P, 256], bf16)
            nc.sync.dma_start(out=wih[:], in_=wih_d[:])
            whh = cp.tile([64, 256], bf16)
            nc.sync.dma_start(out=whh[:], in_=whh_d[:])
            bias = cp.tile([P, 2], f32)
            nc.sync.dma_start(out=bias[:], in_=bias_d[:])
            iota = cp.tile([P, P], f32)
            nc.sync.dma_start(out=iota[:], in_=iota_d[:])
            featloc = cp.tile([P, SLOTS], bf16)
            nc.sync.dma_start(out=featloc[:], in_=floc_d[:])
            iota_ident = cp.tile([P, P], f32)
            make_identity(nc, iota_ident[:])
            nc.gpsimd.load_library(mlp)
            reg_lo = nc.gpsimd.to_reg(NLO_COM * P)
            reg_hi = nc.gpsimd.to_reg(NHI_COM * P)

            qn = 0
            for gi, tls in enumerate(tgroups):
                nlo, nhi = call_sizes[gi]
                base_lo = chunk_of[(tls[0], 0, 0)][0]  # first real lo chunk
                base_hi = base_lo + nlo                # first real hi chunk
                w0 = gi * CALLW
                gblo = gbp.tile([P, NLO_COM, P], bf16, tag="gblo")
                gbhi = gbp.tile([P, NHI_COM, P], bf16, tag="gbhi")
                nc.gpsimd.dma_gather(
                    out_ap=gblo[:], in_ap=featT_d[0:SPLIT, :],
                    idxs_ap=idx16[:, w0:w0 + NLO_COM * 8],
                    num_idxs=NLO_COM * P, num_idxs_reg=reg_lo, elem_size=H,
                    single_packet=False, queue_num=qn % NQ,
                )
                qn += 1
                nc.gpsimd.dma_gather(
                    out_ap=gbhi[:],
                    in_ap=featT_d[SPLIT:NPAD, :],
                    idxs_ap=idx16[:, w0 + NLO_COM * 8:w0 + CALLW],
                    num_idxs=NHI_COM * P, num_idxs_reg=reg_hi, elem_size=H,
                    single_packet=False, queue_num=qn % NQ,
                )
                qn += 1
                for tl in tls:
                    pms = []
                    for e in range(2):
                        lo0, klo = chunk_of[(tl, e, 0)]
                        hi0, khi = chunk_of[(tl, e, 1)]
                        hot = hp.tile([P, KMAX * P], bf16, tag="hot")
                        nc.vector.tensor_tensor(
                            out=hot[:, :klo * P],
                            in0=doff[:, lo0:lo0 + klo].to_broadcast([P, klo, P]),
                            in1=iota[:, None, :].to_broadcast([P, klo, P]),
                            op=mybir.AluOpType.is_equal,
                        )
                        nc.vector.tensor_tensor(
                            out=hot[:, klo * P:(klo + khi) * P],
                            in0=doff[:, hi0:hi0 + khi].to_broadcast([P, khi, P]),
                            in1=iota[:, None, :].to_broadcast([P, khi, P]),
                            op=mybir.AluOpType.is_equal,
                        )
                        pm = psm.tile([P, P], f32, tag=f"m{e}")
                        nchunks = klo + khi
                        for k in range(nchunks):
                            if k < klo:
                                rhs = gblo[:, lo0 - base_lo + k, :]
                            else:
                                rhs = gbhi[:, hi0 - base_hi + k - klo, :]
                            nc.tensor.matmul(
                                out=pm[:], lhsT=hot[:, k * P:(k + 1) * P],
                                rhs=rhs,
                                start=(k == 0), stop=(k == nchunks - 1),
                            )
                        pms.append(pm)
                    # rst in [node, h]: per-partition scale (on ACT), add
                    rnh = ep.tile([P, P], f32, tag="rnh")
                    t1 = ep.tile([P, P], f32, tag="t1")
                    nc.scalar.activation(
                        out=rnh[:], in_=pms[0][:],
                        func=mybir.ActivationFunctionType.Copy,
                        scale=sc[:, tl * 2:tl * 2 + 1],
                    )
                    nc.scalar.activation(
                        out=t1[:], in_=pms[1][:],
                        func=mybir.ActivationFunctionType.Copy,
                        scale=sc[:, tl * 2 + 1:tl * 2 + 2],
                    )
                    nc.vector.tensor_tensor(
                        out=rnh[:], in0=rnh[:], in1=t1[:], op=mybir.AluOpType.add
                    )
                    ptb = pst.tile([P, P], f32, tag="pt")
                    nc.tensor.transpose(
                        out=ptb[:], in_=rnh[:], identity=iota_ident[:]
                    )
                    rstf = ep.tile([P, P], f32, tag="rstf")
                    rstb = ep.tile([P, P], bf16, tag="rstb")
                    nc.scalar.activation(
                        out=rstf[:], in_=ptb[:],
                        func=mybir.ActivationFunctionType.Copy,
                    )
                    nc.vector.tensor_copy(out=rstb[:], in_=ptb[:])
                    # gates
                    pg = []
                    for half in range(2):
                        g_ps = psgp.tile([P, P], f32, tag=f"pg{half}")
                        nc.tensor.matmul(
                            out=g_ps[:], lhsT=wih[:, half * P:(half + 1) * P],
                            rhs=featloc[:, tl * P:(tl + 1) * P],
                            start=True, stop=False,
                        )
                        nc.tensor.matmul(
                            out=g_ps[:], lhsT=whh[:, half * P:(half + 1) * P],
                            rhs=rstb[0:64, :], start=False, stop=True,
                        )
                        pg.append(g_ps)
                    # gates halves: pg0 = [i; f], pg1 = [g; o]
                    sif = ep.tile([P, P], f32, tag="sif")
                    nc.scalar.activation(
                        out=sif[:], in_=pg[0][:],
                        func=mybir.ActivationFunctionType.Sigmoid, bias=bias[:, 0:1],
                    )
                    sog = ep.tile([P, P], f32, tag="sog")
                    nc.scalar.activation(
                        out=sog[0:64, :], in_=pg[1][0:64, :],
                        func=mybir.ActivationFunctionType.Tanh, bias=bias[0:64, 1:2],
                    )
                    nc.scalar.activation(
                        out=sog[64:128, :], in_=pg[1][64:128, :],
                        func=mybir.ActivationFunctionType.Sigmoid,
                        bias=bias[64:128, 1:2],
                    )
                    outsb = ep.tile([P, P], f32, tag="outsb")
                    t2 = ep.tile([64, P], f32, tag="t2")
                    tt = ep.tile([P, P], f32, tag="tt")
                    nc.vector.tensor_tensor(
                        out=t2[:], in0=sif[0:64, :], in1=sog[0:64, :],
                        op=mybir.AluOpType.mult,
                    )
                    nc.vector.tensor_copy(out=tt[64:128, :], in_=t2[:])
                    nc.vector.tensor_tensor(
                        out=outsb[64:128, :], in0=sif[64:128, :],
                        in1=rstf[64:128, :], op=mybir.AluOpType.mult,
                    )
                    nc.vector.tensor_tensor(
                        out=outsb[64:128, :], in0=outsb[64:128, :],
                        in1=tt[64:128, :], op=mybir.AluOpType.add,
                    )
                    nc.scalar.activation(
                        out=tt[64:128, :], in_=outsb[64:128, :],
                        func=mybir.ActivationFunctionType.Tanh,
                    )
                    nc.vector.tensor_tensor(
                        out=tt[64:128, :], in0=sog[64:128, :], in1=tt[64:128, :],
                        op=mybir.AluOpType.mult,
                    )
                    nc.vector.tensor_copy(out=outsb[0:64, :], in_=tt[64:128, :])
                    nc.sync.dma_start(
                        out=outT_d[:, tl * P:(tl + 1) * P], in_=outsb[:]
                    )
    _split_multi_waits(nc, mybir)
    lower_extended_insts(nc)
    return nc


def kernel(feat, src0, dst0, src1, dst1, W_ih, W_hh, b_ih, b_hh):
    global LAST_EXEC_NS
    feat = np.asarray(feat, np.float32)
    src0 = np.asarray(src0, np.int64); dst0 = np.asarray(dst0, np.int64)
    src1 = np.asarray(src1, np.int64); dst1 = np.asarray(dst1, np.int64)
    per_core, shared, node_of_slot, meta = _host_prep(
        feat, src0, dst0, src1, dst1,
        np.asarray(W_ih, np.float32), np.asarray(W_hh, np.float32),
        np.asarray(b_ih, np.float32), np.asarray(b_hh, np.float32),
    )
    nc = _build_nc(meta)
    in_maps = [{**shared, **pc} for pc in per_core]
    from concourse.bass_utils import run_bass_kernel_spmd
    res = run_bass_kernel_spmd(nc, in_maps, list(range(NCORES)), trace=TRACE)
    LAST_EXEC_NS = res.exec_time_ns
    out = np.zeros((N_NODES, H), np.float32)
    for c in range(NCORES):
        oT = res.results[c]["outT"]          # [128, SLOTS]
        valid = node_of_slot[c] >= 0
        nodes = node_of_slot[c][valid]
        blk = oT.T[valid]                    # [n, 128]: cols 0:64=h1, 64:128=c1
        out[nodes] = blk
    return out


# revision 30
# speedup vs baseline: 1.1475x; 1.1475x over previous
"""Trainium2 Bass kernel for nn_LstmConv (GNN message passing + LSTMCell).

Sharding: dst nodes load-balanced across 8 cores (permuted into 49 tiles of
128 slots per core). Per core, edge-source rows are fetched from an HBM bf16
feat table with batched InstDMAGatherAnt calls (one lo/hi half-table pair per
tile group, spread over the 4 SWDGE queues so descriptor emission runs on all
8 Q7 cores). Segment-mean is a one-hot PE matmul per 128-edge chunk; the
LSTMCell runs as two PE matmuls + ACT/DVE epilogue per tile. Output is
written transposed and reassembled on the host.
"""

import sys, os

sys.path.insert(0, "/opt/trn_rl_repo")
sys.path.insert(0, os.path.dirname(os.path.abspath(__file__)))

import numpy as np
from ml_dtypes import bfloat16

N_NODES = 50000
N_EDGES = 800000
H = 128
MSG = 64
P = 128
NCORES = 8
TILES = 49
SLOTS = TILES * P          # 6272 per core
SPLIT = 32768              # int16 index range split for the gather table
NPAD = N_NODES + 4         # featT table rows (spare zero rows)
GBT = int(__import__("os").environ.get("KGBT", "2"))                    # tiles per gather group
NQ = int(__import__("os").environ.get("KNQ", "4"))                     # SWDGE queues

LAST_EXEC_NS = None
TRACE = False


def _wrap16(idx, width):
    """Pack linear index list into [128, width] (idx j at [j%16, j//16],
    replicated across the 8 gpsimd cores)."""
    out = np.zeros((P, width), np.uint16)
    n = len(idx)
    cols = (n + 15) // 16
    blk = np.zeros((16, width), np.uint16)
    flat = np.zeros(cols * 16, np.uint16)
    flat[:n] = np.asarray(idx).astype(np.int16).view(np.uint16)
    blk[:, :cols] = flat.reshape(cols, 16).T
    out[:] = np.tile(blk, (8, 1))
    return out


def _host_prep(feat, src0, dst0, src1, dst1, W_ih, W_hh, b_ih, b_hh):
    deg0 = np.bincount(dst0, minlength=N_NODES)
    deg1 = np.bincount(dst1, minlength=N_NODES)
    w = deg0 + deg1

    # snake-assign nodes (sorted by degree desc) into 392 tiles of <=128
    n_tiles_g = NCORES * TILES
    order = np.argsort(-w, kind="stable")
    tile_of_node = np.empty(N_NODES, np.int32)
    pos_in_tile = np.empty(N_NODES, np.int32)
    tcnt = np.zeros(n_tiles_g, np.int32)
    idx = 0
    fwd = True
    while idx < N_NODES:
        rng = range(n_tiles_g) if fwd else range(n_tiles_g - 1, -1, -1)
        for t in rng:
            if idx >= N_NODES:
                break
            if tcnt[t] < P:
                tile_of_node[order[idx]] = t
                pos_in_tile[order[idx]] = tcnt[t]
                tcnt[t] += 1
                idx += 1
        fwd = not fwd

    # balance tiles over cores by weight: snake over tiles sorted by weight
    tile_w = np.zeros(n_tiles_g, np.int64)
    np.add.at(tile_w, tile_of_node, w)
    torder = np.argsort(-tile_w, kind="stable")
    core_of_tile = np.empty(n_tiles_g, np.int32)
    tl_of_tile = np.empty(n_tiles_g, np.int32)
    k = 0
    fwd = True
    for rnd in range(TILES):
        cr = range(NCORES) if fwd else range(NCORES - 1, -1, -1)
        for c in cr:
            core_of_tile[torder[k]] = c
            tl_of_tile[torder[k]] = rnd
            k += 1
        fwd = not fwd

    core_of_node = core_of_tile[tile_of_node]
    slot_of_node = tl_of_tile[tile_of_node] * P + pos_in_tile  # slot within core

    # node_of_slot per core (-1 = ghost)
    node_of_slot = -np.ones((NCORES, SLOTS), np.int64)
    node_of_slot[core_of_node, slot_of_node] = np.arange(N_NODES)

    # per-node combined scales a_e = 1/max(cnt_e,1) * 1/max(has0+has1,1)
    has0 = (deg0 > 0).astype(np.float32)
    has1 = (deg1 > 0).astype(np.float32)
    invc = 1.0 / np.maximum(has0 + has1, 1.0)
    a0 = invc / np.maximum(deg0, 1.0)
    a1 = invc / np.maximum(deg1, 1.0)

    # per-core per-(tile, etype, half) edge groups (half: src<SPLIT / >=)
    groups = {}   # (core, tl, e, h) -> (srcs, slot_pos)
    for e, (src, dst) in enumerate(((src0, dst0), (src1, dst1))):
        c = core_of_node[dst]
        s = slot_of_node[dst]
        hl = (src >= SPLIT).astype(np.int64)
        key = (((c * TILES + s // P) * 2 + hl) * P + (s % P)).astype(np.int64)
        o = np.argsort(key, kind="stable")
        src_s, pp_s = src[o], (s % P)[o]
        gkey = ((c[o] * TILES + (s // P)[o]) * 2 + hl[o])
        bounds = np.searchsorted(gkey, np.arange(NCORES * TILES * 2 + 1))
        for g in range(NCORES * TILES * 2):
            lo, hi = bounds[g], bounds[g + 1]
            gg = g // 2
            groups[(gg // TILES, gg % TILES, e, g % 2)] = (src_s[lo:hi], pp_s[lo:hi])

    # common chunk counts per (tl, e, half): max over cores, in 128-chunks
    K = np.zeros((TILES, 2, 2), np.int32)
    for tl in range(TILES):
        for e in range(2):
            for hf in range(2):
                m = max(len(groups[(c, tl, e, hf)][0]) for c in range(NCORES))
                K[tl, e, hf] = max((m + 127) // 128, 1)

    # gather groups of GBT tiles; per group: lo call (all tl,e half=0) then hi.
    # Snake-pack tiles into groups by total chunk count so per-group chunk
    # sums are near-equal (minimizes the uniform-call-size padding).
    ngroups = (TILES + GBT - 1) // GBT
    tw = [(int(K[tl].sum()), tl) for tl in range(TILES)]
    tw.sort(reverse=True)
    gsum = [0] * ngroups
    gcnt = [0] * ngroups
    tg = [[] for _ in range(ngroups)]
    for wgt, tl in tw:
        best = min((g for g in range(ngroups) if gcnt[g] < GBT),
                   key=lambda g: gsum[g])
        tg[best].append(tl)
        gsum[best] += wgt
        gcnt[best] += 1
    tgroups = [sorted(g) for g in tg]

    # real-chunk order = for each tgroup: [lo chunks of (tl,e)...] +
    # [hi chunks of (tl,e)...]; doff col == global real-chunk idx.
    # All lo calls share num_idxs=NLO_COM*128 (one register), likewise hi;
    # the per-group shortfall is -1 idx padding, self-trimmed by the ucode.
    chunk_of = {}   # (tl, e, hf) -> (start_chunk, K)
    call_sizes = []  # per group: (n_real_lo_chunks, n_real_hi_chunks)
    cidx = 0
    for tls in tgroups:
        nlo = 0
        for tl in tls:
            for e in range(2):
                chunk_of[(tl, e, 0)] = (cidx, int(K[tl, e, 0]))
                cidx += K[tl, e, 0]
                nlo += int(K[tl, e, 0])
        nhi = 0
        for tl in tls:
            for e in range(2):
                chunk_of[(tl, e, 1)] = (cidx, int(K[tl, e, 1]))
                cidx += K[tl, e, 1]
                nhi += int(K[tl, e, 1])
        call_sizes.append((nlo, nhi))
    CTOT = cidx
    NLO_COM = max(n for n, _ in call_sizes)
    NHI_COM = max(n for _, n in call_sizes)

    # per-core data: idx16 (uniform-size call blocks), doff, scales, featloc
    per_core = []
    CALLW = (NLO_COM + NHI_COM) * 8   # wrapped idx cols per group
    WTOT = len(tgroups) * CALLW
    for c in range(NCORES):
        doff = np.full((P, CTOT), 255.0, np.float32)
        idxlin = np.zeros(CTOT * P, np.int64)
        for tl in range(TILES):
            for e in range(2):
                for hf in range(2):
                    srcs, pps = groups[(c, tl, e, hf)]
                    st, k = chunk_of[(tl, e, hf)]
                    n = len(srcs)
                    a = np.zeros(k * P, np.int64)
                    a[:n] = srcs - (SPLIT if hf else 0)
                    idxlin[st * P:(st + k) * P] = a
                    d = np.full(k * P, 255.0, np.float32)
                    d[:n] = pps
                    doff[:, st:st + k] = d.reshape(k, P).T
        idx16 = np.zeros((P, WTOT), np.int16)
        cbase = 0
        for gi, tls in enumerate(tgroups):
            nlo, nhi = call_sizes[gi]
            PADIDX = -1 if os.environ.get("KPAD", "neg") == "neg" else 0
            lo = np.full(NLO_COM * P, PADIDX, np.int64)
            lo[:nlo * P] = idxlin[cbase * P:(cbase + nlo) * P]
            hi = np.full(NHI_COM * P, PADIDX, np.int64)
            hi[:nhi * P] = idxlin[(cbase + nlo) * P:(cbase + nlo + nhi) * P]
            w0 = gi * CALLW
            idx16[:, w0:w0 + NLO_COM * 8] = _wrap16(
                lo.astype(np.int16), NLO_COM * 8)
            idx16[:, w0 + NLO_COM * 8:w0 + CALLW] = _wrap16(
                hi.astype(np.int16), NHI_COM * 8)
            cbase += nlo + nhi

        sc = np.zeros((P, TILES * 2), np.float32)
        av0 = np.where(node_of_slot[c] >= 0, a0[np.maximum(node_of_slot[c], 0)], 0.0)
        av1 = np.where(node_of_slot[c] >= 0, a1[np.maximum(node_of_slot[c], 0)], 0.0)
        for tl in range(TILES):
            sc[:, tl * 2] = av0[tl * P:(tl + 1) * P]
            sc[:, tl * 2 + 1] = av1[tl * P:(tl + 1) * P]
        sl = node_of_slot[c]
        floc = np.zeros((SLOTS, H), np.float32)
        floc[sl >= 0] = feat[sl[sl >= 0]]
        per_core.append(dict(idx16=idx16, doff=doff, scales=sc,
                             featloc=floc.T.astype(bfloat16).copy()))

    # feat table [NPAD, 128] bf16 (row 0 doubles as the pad row: its
    # contribution is killed by doff=255 -> hot col 0)
    featT = np.zeros((NPAD, H), np.float32)
    featT[:N_NODES] = feat
    featT = featT.astype(bfloat16)

    # gate order [i, f, g, o] (PyTorch native)
    wih = W_ih.T.astype(bfloat16).copy()              # [128, 256]
    whh = W_hh.T.astype(bfloat16).copy()              # [64, 256]
    bt = (b_ih + b_hh).astype(np.float32)
    biasT = np.stack([bt[:128], bt[128:]], axis=1).copy()  # [128, 2]
    iota = np.tile(np.arange(P, dtype=np.float32)[None, :], (P, 1))

    shared = dict(featT=featT, wih=wih, whh=whh, biasT=biasT, iota=iota)
    meta = (tgroups, chunk_of, call_sizes, CTOT, WTOT, NLO_COM, NHI_COM)
    return per_core, shared, node_of_slot, meta


_WS = [0]


def _split_multi_waits(nc, mybir, max_waits=1):
    """This container's walrus rejects >1 sync wait per instruction; split
    extra waits onto single-wait NoOps placed just before the instruction."""
    for fn in nc.m.functions:
        for bb in fn.blocks:
            new = []
            for ins in bb.instructions:
                si = ins.sync_info
                if si is not None and len(si.on_wait) > max_waits:
                    waits = list(si.on_wait)
                    for w in waits[:-max_waits]:
                        _WS[0] += 1
                        nop = mybir.InstNoOp(
                            name=f"I-waitsplit-{_WS[0]}", ins=[], outs=[]
                        )
                        nop.engine = ins.engine
                        nop.sync_info = mybir.SyncInfo(on_wait=[w], on_update=[])
                        new.append(nop)
                    si.on_wait = waits[-max_waits:]
                new.append(ins)
            bb.instructions[:] = new


def _build_nc(meta):
    from concourse import bass, mybir, tile
    from concourse.masks import make_identity
    from concourse.library_config import mlp
    from concourse.library_overlay import lower_extended_insts

    tgroups, chunk_of, call_sizes, CTOT, WTOT, NLO_COM, NHI_COM = meta
    CALLW = (NLO_COM + NHI_COM) * 8
    f32, bf16, i16 = mybir.dt.float32, mybir.dt.bfloat16, mybir.dt.int16
    nc = bass.Bass(num_swdge_queues=NQ)
    featT_d = nc.declare_dram_parameter("featT", [NPAD, H], bf16, isOutput=False)
    idx16_d = nc.declare_dram_parameter("idx16", [P, WTOT], i16, isOutput=False)
    doff_d = nc.declare_dram_parameter("doff", [P, CTOT], f32, isOutput=False)
    sc_d = nc.declare_dram_parameter("scales", [P, TILES * 2], f32, isOutput=False)
    wih_d = nc.declare_dram_parameter("wih", [P, 256], bf16, isOutput=False)
    whh_d = nc.declare_dram_parameter("whh", [64, 256], bf16, isOutput=False)
    bias_d = nc.declare_dram_parameter("biasT", [P, 2], f32, isOutput=False)
    iota_d = nc.declare_dram_parameter("iota", [P, P], f32, isOutput=False)
    floc_d = nc.declare_dram_parameter("featloc", [P, SLOTS], bf16, isOutput=False)
    outT_d = nc.declare_dram_parameter("outT", [P, SLOTS], f32, isOutput=True)

    GCMAX = NLO_COM + NHI_COM
    KMAX = max(chunk_of[(tl, e, 0)][1] + chunk_of[(tl, e, 1)][1]
               for tl in range(TILES) for e in range(2))

    with tile.TileContext(nc) as tc:
        with (
            tc.tile_pool(name="const", bufs=1) as cp,
            tc.tile_pool(name="gb", bufs=4) as gbp,
            tc.tile_pool(name="hot", bufs=3) as hp,
            tc.tile_pool(name="ep", bufs=2) as ep,
            tc.tile_pool(name="psm", bufs=2, space="PSUM") as psm,
            tc.tile_pool(name="pst", bufs=1, space="PSUM") as pst,
            tc.tile_pool(name="psg", bufs=1, space="PSUM") as psgp,
        ):
            idx16 = cp.tile([P, WTOT], i16)
            nc.sync.dma_start(out=idx16[:], in_=idx16_d[:])
            doff = cp.tile([P, CTOT], f32)
            nc.sync.dma_start(out=doff[:], in_=doff_d[:])
            sc = cp.tile([P, TILES * 2], f32)
            nc.sync.dma_start(out=sc[:], in_=sc_d[:])
            wih = cp.tile([P, 256], bf16)
            nc.sync.dma_start(out=wih[:], in_=wih_d[:])
            whh = cp.tile([64, 256], bf16)
            nc.sync.dma_start(out=whh[:], in_=whh_d[:])
            bias = cp.tile([P, 2], f32)
            nc.sync.dma_start(out=bias[:], in_=bias_d[:])
            iota = cp.tile([P, P], f32)
            nc.sync.dma_start(out=iota[:], in_=iota_d[:])
            featloc = cp.tile([P, SLOTS], bf16)
            nc.sync.dma_start(out=featloc[:], in_=floc_d[:])
            iota_ident = cp.tile([P, P], f32)
            make_identity(nc, iota_ident[:])
            nc.gpsimd.load_library(mlp)
            reg_lo = nc.gpsimd.to_reg(NLO_COM * P)
            reg_hi = nc.gpsimd.to_reg(NHI_COM * P)

            qn = 0
            for gi, tls in enumerate(tgroups):
                nlo, nhi = call_sizes[gi]
                base_lo = chunk_of[(tls[0], 0, 0)][0]  # first real lo chunk
                base_hi = base_lo + nlo                # first real hi chunk
                w0 = gi * CALLW
                gblo = gbp.tile([P, NLO_COM, P], bf16, tag="gblo")
                gbhi = gbp.tile([P, NHI_COM, P], bf16, tag="gbhi")
                nc.gpsimd.dma_gather(
                    out_ap=gblo[:], in_ap=featT_d[0:SPLIT, :],
                    idxs_ap=idx16[:, w0:w0 + NLO_COM * 8],
                    num_idxs=NLO_COM * P, num_idxs_reg=reg_lo, elem_size=H,
                    single_packet=False, queue_num=qn % NQ,
                )
                qn += 1
                nc.gpsimd.dma_gather(
                    out_ap=gbhi[:],
                    in_ap=featT_d[SPLIT:NPAD, :],
                    idxs_ap=idx16[:, w0 + NLO_COM * 8:w0 + CALLW],
                    num_idxs=NHI_COM * P, num_idxs_reg=reg_hi, elem_size=H,
                    single_packet=False, queue_num=qn % NQ,
                )
                qn += 1
                for tl in tls:
                    pms = []
                    for e in range(2):
                        lo0, klo = chunk_of[(tl, e, 0)]
                        hi0, khi = chunk_of[(tl, e, 1)]
                        hot = hp.tile([P, KMAX * P], bf16, tag="hot")
                        nc.vector.tensor_tensor(
                            out=hot[:, :klo * P],
                            in0=doff[:, lo0:lo0 + klo].to_broadcast([P, klo, P]),
                            in1=iota[:, None, :].to_broadcast([P, klo, P]),
                            op=mybir.AluOpType.is_equal,
                        )
                        nc.vector.tensor_tensor(
                            out=hot[:, klo * P:(klo + khi) * P],
                            in0=doff[:, hi0:hi0 + khi].to_broadcast([P, khi, P]),
                            in1=iota[:, None, :].to_broadcast([P, khi, P]),
                            op=mybir.AluOpType.is_equal,
                        )
                        pm = psm.tile([P, P], f32, tag=f"m{e}")
                        nchunks = klo + khi
                        for k in range(nchunks):
                            if k < klo:
                                rhs = gblo[:, lo0 - base_lo + k, :]
                            else:
                                rhs = gbhi[:, hi0 - base_hi + k - klo, :]
                            nc.tensor.matmul(
                                out=pm[:], lhsT=hot[:, k * P:(k + 1) * P],
                                rhs=rhs,
                                start=(k == 0), stop=(k == nchunks - 1),
                            )
                        pms.append(pm)
                    # rst in [node, h]: per-partition scale, then transpose
                    rnh = ep.tile([P, P], f32, tag="rnh")
                    t1 = ep.tile([P, P], f32, tag="t1")
                    nc.vector.tensor_scalar(
                        out=rnh[:], in0=pms[0][:],
                        scalar1=sc[:, tl * 2:tl * 2 + 1], scalar2=None,
                        op0=mybir.AluOpType.mult,
                    )
                    nc.vector.tensor_scalar(
                        out=t1[:], in0=pms[1][:],
                        scalar1=sc[:, tl * 2 + 1:tl * 2 + 2], scalar2=None,
                        op0=mybir.AluOpType.mult,
                    )
                    nc.vector.tensor_tensor(
                        out=rnh[:], in0=rnh[:], in1=t1[:], op=mybir.AluOpType.add
                    )
                    ptb = pst.tile([P, P], f32, tag="pt")
                    nc.tensor.transpose(
                        out=ptb[:], in_=rnh[:], identity=iota_ident[:]
                    )
                    rstf = ep.tile([P, P], f32, tag="rstf")
                    rstb = ep.tile([P, P], bf16, tag="rstb")
                    nc.vector.tensor_copy(out=rstf[:], in_=ptb[:])
                    nc.vector.tensor_copy(out=rstb[:], in_=ptb[:])
                    # gates
                    pg = []
                    for half in range(2):
                        g_ps = psgp.tile([P, P], f32, tag=f"pg{half}")
                        nc.tensor.matmul(
                            out=g_ps[:], lhsT=wih[:, half * P:(half + 1) * P],
                            rhs=featloc[:, tl * P:(tl + 1) * P],
                            start=True, stop=False,
                        )
                        nc.tensor.matmul(
                            out=g_ps[:], lhsT=whh[:, half * P:(half + 1) * P],
                            rhs=rstb[0:64, :], start=False, stop=True,
                        )
                        pg.append(g_ps)
                    # gates halves: pg0 = [i; f], pg1 = [g; o]
                    sif = ep.tile([P, P], f32, tag="sif")
                    nc.scalar.activation(
                        out=sif[:], in_=pg[0][:],
                        func=mybir.ActivationFunctionType.Sigmoid, bias=bias[:, 0:1],
                    )
                    sog = ep.tile([P, P], f32, tag="sog")
                    nc.scalar.activation(
                        out=sog[0:64, :], in_=pg[1][0:64, :],
                        func=mybir.ActivationFunctionType.Tanh, bias=bias[0:64, 1:2],
                    )
                    nc.scalar.activation(
                        out=sog[64:128, :], in_=pg[1][64:128, :],
                        func=mybir.ActivationFunctionType.Sigmoid,
                        bias=bias[64:128, 1:2],
                    )
                    outsb = ep.tile([P, P], f32, tag="outsb")
                    t2 = ep.tile([64, P], f32, tag="t2")
                    tt = ep.tile([P, P], f32, tag="tt")
                    nc.vector.tensor_tensor(
                        out=t2[:], in0=sif[0:64, :], in1=sog[0:64, :],
                        op=mybir.AluOpType.mult,
                    )
                    nc.vector.tensor_copy(out=tt[64:128, :], in_=t2[:])
                    nc.vector.tensor_tensor(
                        out=outsb[64:128, :], in0=sif[64:128, :],
                        in1=rstf[64:128, :], op=mybir.AluOpType.mult,
                    )
                    nc.vector.tensor_tensor(
                        out=outsb[64:128, :], in0=outsb[64:128, :],
                        in1=tt[64:128, :], op=mybir.AluOpType.add,
                    )
                    nc.scalar.activation(
                        out=tt[64:128, :], in_=outsb[64:128, :],
                        func=mybir.ActivationFunctionType.Tanh,
                    )
                    nc.vector.tensor_tensor(
                        out=tt[64:128, :], in0=sog[64:128, :], in1=tt[64:128, :],
                        op=mybir.AluOpType.mult,
                    )
                    nc.vector.tensor_copy(out=outsb[0:64, :], in_=tt[64:128, :])
                    nc.sync.dma_start(
                        out=outT_d[:, tl * P:(tl + 1) * P], in_=outsb[:]
                    )
    _split_multi_waits(nc, mybir)
    lower_extended_insts(nc)
    return nc


def kernel(feat, src0, dst0, src1, dst1, W_ih, W_hh, b_ih, b_hh):
    global LAST_EXEC_NS
    feat = np.asarray(feat, np.float32)
    src0 = np.asarray(src0, np.int64); dst0 = np.asarray(dst0, np.int64)
    src1 = np.asarray(src1, np.int64); dst1 = np.asarray(dst1, np.int64)
    per_core, shared, node_of_slot, meta = _host_prep(
        feat, src0, dst0, src1, dst1,
        np.asarray(W_ih, np.float32), np.asarray(W_hh, np.float32),
        np.asarray(b_ih, np.float32), np.asarray(b_hh, np.float32),
    )
    nc = _build_nc(meta)
    in_maps = [{**shared, **pc} for pc in per_core]
    from concourse.bass_utils import run_bass_kernel_spmd
    res = run_bass_kernel_spmd(nc, in_maps, list(range(NCORES)), trace=TRACE)
    LAST_EXEC_NS = res.exec_time_ns
    out = np.zeros((N_NODES, H), np.float32)
    for c in range(NCORES):
        oT = res.results[c]["outT"]          # [128, SLOTS]
        valid = node_of_slot[c] >= 0
        nodes = node_of_slot[c][valid]
        blk = oT.T[valid]                    # [n, 128]: cols 0:64=h1, 64:128=c1
        out[nodes] = blk
    return out
